# revision 59
# baseline (speedup 1.0000x reference)
"""Trainium2 Bass kernel for nn_Dynamic_Fusion (gnn_message_passing).

Reference computation (per batch item b):
  scores[n] = sum_{h,m} attn[b,h,n,m]            (argmax invariant to the /H mean)
  t         = argmax_n scores[n]                 (first index on ties)
  a         = depth1_ancestor(t)  in {0,1,4,7}
  update    = points[b,a] + (t!=0) * vectors[b,a-1]
  out[b,v]  = points[b,v] + Fa*update - Fa*sum_{edges e on root->v path} vectors[b,e]

Strategy: pure data parallel over 8 cores (512 batch items each), batch on
the 128 SBUF partitions (4 tiles of 128 per core), z=512 on the free dim.
The path-sum term is computed with a tree recurrence
  T[v] = T[parent(v)] - Fa*vectors[v-1],  T[0] = Fa*update
using fused scalar_tensor_tensor ops (grouped into affine node-slices and
written in place into the vectors tile), then out = T + points in a big
tensor_tensor add (in place into the points tile, which is then stored).
The argmax is done with reduce_max + is_ge + iota-min (replicating argsort
first-max tie-breaking).  Loads and the add+store are split so the store of
nodes 0..10 overlaps the chain tail, and the head rows (update inputs +
chain prefix) land before the bulk.

Per core this moves 57.2 MB HBM traffic (all inputs once + output once,
irreducible) against a ~358 GB/s/core HBM limit -> ~160 us roofline.
Engine busy (cost model): DMA 159 us, DVE 100 us, ACT 11 us; TimelineSim
estimate 164.5 us (DMA ~98% occupied; only fixed first-DMA setup +
last-store receipt remain).  HW-tuned beyond the model:
 - loads on the SP HWDGE ring, stores + consts on the ACT HWDGE ring
   (SWDGE stores measured ~10 us slower on HW: Q7 descriptor generation
   interferes with DVE's shared SBUF port);
 - points triple-buffered (bufs=2 measured +14 us on HW);
 - the first store (rows 0:10) gates on only 5 chain ops + one add and
   needs neither the points tail nor the chain tail: measured against a
   loads+stores-only probe NEFF (no compute), this cut the
   compute-induced DMA stall from ~13 us to ~3 us on HW.
Measured via repeat-loop differencing (R=513): 171.3 us best window
(172.5 us p50), ~173-178 us typical, within ~0.2-3 us of its own
loads+stores-only probe NEFF.  Probe
decomposition: loads-only runs at 346-359 GB/s (the HBM-per-core read
limit), stores-only at ~280 GB/s, and reads/writes strictly serialize
on the DMA path (full probe = loads probe + stores probe exactly), so
the achievable floor on this silicon is ~173 us, not bytes/358 = 160 us.
Single contiguous stores would recover ~3 us of write bandwidth but
cost ~12 us of store gating (measured) — the split wins.  The write
ceiling is the hardware's: stores-only probes via HWDGE vs SWDGE,
split vs single-contiguous, all measure ~290 GB/s within noise.
A probe with fully independent load/store streams (stores enqueued at
t=0, no data dependency) runs ~7 us faster than the gated probe: that
residue is write-stream burstiness from stores entering the DMA queue
only at their compute gates.  The default 4-way add+store split (rows
0:3 / 3:10 / 10:13 / 13:17, each gating on the earliest chain op that
finalizes its rows) recovers ~3-7 us of it on HW vs the 2-way split;
a 5-way split with a tiny early row-0 store measures worse (the
optimum is bracketed).  Loading the vectors tail before the points
tail (chain tail then overlaps the points-tail transfer) is worth a
further ~1-5 us on HW (model-neutral).
"""

import sys

for _p in ("/opt/trn_rl_repo",):
    if _p not in sys.path:
        sys.path.insert(0, _p)

from contextlib import ExitStack

import numpy as np

import concourse.bass as bass  # noqa: F401
import concourse.tile as tile
from concourse import bacc, mybir
from concourse.bass_utils import run_bass_kernel_spmd

F32 = mybir.dt.float32
ALU = mybir.AluOpType
AX = mybir.AxisListType

N_CORES = 8
B_FULL = 4096
B = B_FULL // N_CORES  # 512 batch items per core
NJ = 17  # joints
NE = 16  # edges
Z = 512
H = 8
P = 128  # SBUF partitions = batch tile
NTILES = B // P  # 4

_nc_cache = None


def _build(
    reps=1,
    split_loads=True,
    split_stores=4,
    pts_bufs=3,
    vec_load_engine="sync",
    store_engine="scalar",
    merge_attn=False,
    tail_engine="sync",
    dma_only=False,
    swap_tails=True,
    split_ptail=False,
    attn_engine="sync",
):
    nc = bacc.Bacc("TRN2", target_bir_lowering=False, debug=False, name="dynfusion")

    pts = nc.dram_tensor("points", [B, NJ, Z], F32, kind="ExternalInput")
    vec = nc.dram_tensor("vectors", [B, NE, Z], F32, kind="ExternalInput")
    att = nc.dram_tensor("attn", [B, H, NJ, NJ], F32, kind="ExternalInput")
    fa_pos = nc.dram_tensor("fa_pos", [P, 1], F32, kind="ExternalInput")
    fa_neg = nc.dram_tensor("fa_neg", [P, 1], F32, kind="ExternalInput")
    iota = nc.dram_tensor("iota", [P, NJ], F32, kind="ExternalInput")
    out = nc.dram_tensor("out", [B, NJ, Z], F32, kind="ExternalOutput")

    stt = None  # set below (nc.vector.scalar_tensor_tensor)

    with tile.TileContext(nc) as tc, ExitStack() as ctx:
        consts = ctx.enter_context(tc.tile_pool(name="consts", bufs=1))
        p_pool = ctx.enter_context(tc.tile_pool(name="pts", bufs=pts_bufs))
        v_pool = ctx.enter_context(tc.tile_pool(name="vec", bufs=2))
        a_pool = ctx.enter_context(
            tc.tile_pool(name="attn", bufs=1 if merge_attn else 2)
        )
        u_pool = ctx.enter_context(tc.tile_pool(name="uscr", bufs=2))
        s_pool = ctx.enter_context(tc.tile_pool(name="small", bufs=2))

        stt = nc.vector.scalar_tensor_tensor
        vec_eng = getattr(nc, vec_load_engine)
        store_eng = getattr(nc, store_engine)

        fa_p = consts.tile([P, 1], F32)
        nc.scalar.dma_start(fa_p[:], fa_pos.ap())
        fa_n = consts.tile([P, 1], F32)
        nc.scalar.dma_start(fa_n[:], fa_neg.ap())
        io = consts.tile([P, NJ], F32)
        nc.scalar.dma_start(io[:], iota.ap())

        rep_ctx = tc.For_i(0, reps, 1) if reps > 1 else None
        if rep_ctx is not None:
            rep_ctx.__enter__()

        tail_eng = getattr(nc, tail_engine)
        A_pair = None

        dummy_ind = None
        if dma_only == "indep":
            # loads + stores with NO dependency between them: distinguishes
            # hardware R/W arbitration from load->store gating burstiness
            dummy_ind = consts.tile([P, NJ, Z], F32)
            nc.vector.memset(dummy_ind[:], 1.0)

        if dma_only in ("stores", "stores1"):
            # pure write-bandwidth probe: store one memset tile to every
            # output region (no loads)
            dummy = p_pool.tile([P, NJ, Z], F32)
            nc.vector.memset(dummy[:], 1.0)
            for it in range(NTILES):
                r0 = it * P
                if dma_only == "stores1":
                    store_eng.dma_start(out.ap()[r0 : r0 + P], dummy[:])
                else:
                    store_eng.dma_start(
                        out.ap()[r0 : r0 + P, :10], dummy[:, :10, :]
                    )
                    store_eng.dma_start(
                        out.ap()[r0 : r0 + P, 10:], dummy[:, 10:, :]
                    )

        for it in range(NTILES) if dma_only not in ("stores", "stores1") else []:
            r0 = it * P

            if merge_attn:
                # One DMA per PAIR of tiles (partition p holds rows p and
                # 128+p of the pair's 256-row block), issued on the scalar
                # ring so the SP ring starts with the point/vector heads.
                if it % 2 == 0:
                    A_pair = a_pool.tile([P, 2, H, NJ, NJ], F32)
                    att_view = bass.AP(
                        tensor=att.ap().tensor,
                        offset=r0 * H * NJ * NJ,
                        ap=[
                            [H * NJ * NJ, P],
                            [P * H * NJ * NJ, 2],
                            [1, H * NJ * NJ],
                        ],
                    )
                    nc.scalar.dma_start(
                        A_pair[:].rearrange("p t h n m -> p t (h n m)"), att_view
                    )
                A = A_pair[:, it % 2]
            else:
                A = a_pool.tile([P, H, NJ, NJ], F32)
                getattr(nc, attn_engine).dma_start(A[:], att.ap()[r0 : r0 + P])
            # Split loads: the head rows feed the update selection and the
            # chain prefix (nodes 0..10), so compute can start before the
            # tails arrive.
            V = v_pool.tile([P, NE, Z], F32)
            Pt = p_pool.tile([P, NJ, Z], F32)
            if split_loads:
                nc.sync.dma_start(Pt[:, :10, :], pts.ap()[r0 : r0 + P, :10])
                vec_eng.dma_start(V[:, :10, :], vec.ap()[r0 : r0 + P, :10])
                if swap_tails:
                    # vec tail first: the chain tail (gating the late store
                    # chunks) runs during the points-tail transfer
                    tail_eng.dma_start(V[:, 10:, :], vec.ap()[r0 : r0 + P, 10:])
                    if split_ptail:
                        # rows 10:13 un-gate the third store chunk early
                        tail_eng.dma_start(
                            Pt[:, 10:13, :], pts.ap()[r0 : r0 + P, 10:13]
                        )
                        tail_eng.dma_start(
                            Pt[:, 13:, :], pts.ap()[r0 : r0 + P, 13:]
                        )
                    else:
                        tail_eng.dma_start(
                            Pt[:, 10:, :], pts.ap()[r0 : r0 + P, 10:]
                        )
                else:
                    tail_eng.dma_start(Pt[:, 10:, :], pts.ap()[r0 : r0 + P, 10:])
                    tail_eng.dma_start(V[:, 10:, :], vec.ap()[r0 : r0 + P, 10:])
            else:
                nc.sync.dma_start(Pt[:], pts.ap()[r0 : r0 + P])
                vec_eng.dma_start(V[:], vec.ap()[r0 : r0 + P])

            if dma_only:
                # bandwidth probe: same loads (+stores unless loads-only),
                # no compute
                if dma_only == "indep":
                    store_eng.dma_start(
                        out.ap()[r0 : r0 + P, :10], dummy_ind[:, :10, :]
                    )
                    store_eng.dma_start(
                        out.ap()[r0 : r0 + P, 10:], dummy_ind[:, 10:, :]
                    )
                elif dma_only != "loads":
                    store_eng.dma_start(out.ap()[r0 : r0 + P, :10], Pt[:, :10, :])
                    store_eng.dma_start(out.ap()[r0 : r0 + P, 10:], Pt[:, 10:, :])
                continue

            # --- scores[n] = sum over (h, m): one XY-reduce on [p, n, h, m] view
            sc = s_pool.tile([P, NJ], F32)
            nc.vector.tensor_reduce(
                sc[:], A[:].rearrange("p h n m -> p n h m"), axis=AX.XY, op=ALU.add
            )
            # --- argmax with first-index tie-break
            mx = s_pool.tile([P, 1], F32)
            nc.vector.tensor_reduce(mx[:], sc[:], axis=AX.X, op=ALU.max)
            eq = s_pool.tile([P, NJ], F32)
            nc.vector.tensor_scalar(eq[:], sc[:], mx[:], None, ALU.is_ge)
            msk = s_pool.tile([P, NJ], F32)
            stt(msk[:], eq[:], -1.0e4, io[:], op0=ALU.mult, op1=ALU.add)
            tb = s_pool.tile([P, 1], F32)
            nc.vector.tensor_reduce(tb[:], msk[:], axis=AX.X, op=ALU.min)
            # tb currently = argmax - 1e4; compare against shifted thresholds
            # (avoids an extra +1e4 op): t < x  <=>  tb < x - 1e4
            c0 = s_pool.tile([P, 1], F32)
            nc.vector.tensor_scalar(c0[:], tb[:], 0.5 - 1.0e4, None, ALU.is_lt)
            c3 = s_pool.tile([P, 1], F32)
            nc.vector.tensor_scalar(c3[:], tb[:], 3.5 - 1.0e4, None, ALU.is_lt)
            c6 = s_pool.tile([P, 1], F32)
            nc.vector.tensor_scalar(c6[:], tb[:], 6.5 - 1.0e4, None, ALU.is_lt)
            s1 = s_pool.tile([P, 1], F32)
            stt(s1[:], c0[:], -1.0, c3[:], op0=ALU.mult, op1=ALU.add)  # c3-c0
            s4 = s_pool.tile([P, 1], F32)
            stt(s4[:], c3[:], -1.0, c6[:], op0=ALU.mult, op1=ALU.add)  # c6-c3
            s7 = s_pool.tile([P, 1], F32)
            nc.vector.tensor_scalar(s7[:], c6[:], -1.0, 1.0, ALU.mult, ALU.add)

            # --- update selection.  The chain values T[v] are stored in
            # place in the vectors tile (slot of T[v] is V[:, v-1, :]; each
            # chain op consumes its raw vector row in the same instruction),
            # with T[0] in a small dedicated tile.
            t0 = u_pool.tile([P, Z], F32)
            u1 = u_pool.tile([P, Z], F32)
            u4 = u_pool.tile([P, Z], F32)
            u7 = u_pool.tile([P, Z], F32)
            nc.scalar.mul(t0[:], Pt[:, 0, :], c0[:])
            nc.scalar.mul(u1[:], Pt[:, 1, :], s1[:])
            nc.scalar.mul(u4[:], Pt[:, 4, :], s4[:])
            nc.scalar.mul(u7[:], Pt[:, 7, :], s7[:])
            nc.vector.tensor_add(t0[:], t0[:], u1[:])
            nc.vector.tensor_add(u4[:], u4[:], u7[:])
            nc.vector.tensor_add(t0[:], t0[:], u4[:])  # = selected point row
            stt(t0[:], V[:, 0, :], s1[:], t0[:], op0=ALU.mult, op1=ALU.add)
            stt(t0[:], V[:, 3, :], s4[:], t0[:], op0=ALU.mult, op1=ALU.add)
            stt(t0[:], V[:, 6, :], s7[:], t0[:], op0=ALU.mult, op1=ALU.add)
            # T[0] = Fa * update
            nc.vector.tensor_scalar(t0[:], t0[:], fa_p[:], None, ALU.mult)

            # --- downward tree chain: T[v] = T[parent] - Fa*V[v-1], written
            # into V[v-1]; grouped into affine strided slices where parents
            # line up.
            def chain(rows, par):
                stt(rows, rows, fa_n[:], par, op0=ALU.mult, op1=ALU.add)

            # --- out = T + points (in place into the points tile), then store.
            if split_stores in (4, 5):
                # 4/5-way spread of the write stream: each add+store chunk
                # gates on the earliest chain op that finalizes its rows,
                # smoothing store enqueue across the tile's compute timeline.
                nc.vector.tensor_add(Pt[:, 0, :], t0[:], Pt[:, 0, :])
                if split_stores == 5:
                    # row 0 is final before any chain op — store immediately
                    store_eng.dma_start(out.ap()[r0 : r0 + P, :1], Pt[:, :1, :])
                chain(V[:, 0, :], t0[:])  # T1
                chain(V[:, 3, :], t0[:])  # T4
                chain(V[:, 6, :], t0[:])  # T7
                chain(V[:, 1:8:3, :], V[:, 0:7:3, :])  # T{2,5,8}
                # rows 1:3 need T1 (op1), T2 (op4)
                pa1 = Pt[:, 1:3, :].rearrange("p a b -> p (a b)")
                nc.vector.tensor_add(
                    pa1, V[:, 0:2, :].rearrange("p a b -> p (a b)"), pa1
                )
                if split_stores == 5:
                    store_eng.dma_start(out.ap()[r0 : r0 + P, 1:3], Pt[:, 1:3, :])
                else:
                    store_eng.dma_start(out.ap()[r0 : r0 + P, :3], Pt[:, :3, :])

                chain(V[:, 2:9:3, :], V[:, 1:8:3, :])  # T{3,6,9}
                # rows 3:10 need T3..T9 (op5)
                pa2 = Pt[:, 3:10, :].rearrange("p a b -> p (a b)")
                nc.vector.tensor_add(
                    pa2, V[:, 2:9, :].rearrange("p a b -> p (a b)"), pa2
                )
                store_eng.dma_start(out.ap()[r0 : r0 + P, 3:10], Pt[:, 3:10, :])

                chain(V[:, 9, :], V[:, 8, :])  # T10
                chain(V[:, 10, :], V[:, 7, :])  # T11
                chain(V[:, 13, :], V[:, 7, :])  # T14
                chain(V[:, 11:15:3, :], V[:, 10:14:3, :])  # T{12,15}
                # rows 10:13 need T10 (op6), T11 (op7), T12 (op9)
                pb1 = Pt[:, 10:13, :].rearrange("p a b -> p (a b)")
                nc.vector.tensor_add(
                    pb1, V[:, 9:12, :].rearrange("p a b -> p (a b)"), pb1
                )
                store_eng.dma_start(out.ap()[r0 : r0 + P, 10:13], Pt[:, 10:13, :])

                chain(V[:, 12:16:3, :], V[:, 11:15:3, :])  # T{13,16}
                # rows 13:17 need T13,T16 (op10), T14 (op8), T15 (op9)
                pb2 = Pt[:, 13:17, :].rearrange("p a b -> p (a b)")
                nc.vector.tensor_add(
                    pb2, V[:, 12:16, :].rearrange("p a b -> p (a b)"), pb2
                )
                store_eng.dma_start(out.ap()[r0 : r0 + P, 13:], Pt[:, 13:, :])
            elif split_stores:
                # Row 0 is final right after the update phase; rows 1..9
                # (T1..T9) after five chain ops.  Store rows 0:10 then, so
                # the store needs neither the points tail nor the chain
                # tail; rows 10:17 are added + stored at the end.
                nc.vector.tensor_add(Pt[:, 0, :], t0[:], Pt[:, 0, :])
                chain(V[:, 0, :], t0[:])  # T1
                chain(V[:, 3, :], t0[:])  # T4
                chain(V[:, 6, :], t0[:])  # T7
                chain(V[:, 1:8:3, :], V[:, 0:7:3, :])  # T{2,5,8}
                chain(V[:, 2:9:3, :], V[:, 1:8:3, :])  # T{3,6,9}
                pf1 = Pt[:, 1:10, :].rearrange("p a b -> p (a b)")
                nc.vector.tensor_add(
                    pf1, V[:, 0:9, :].rearrange("p a b -> p (a b)"), pf1
                )
                store_eng.dma_start(out.ap()[r0 : r0 + P, :10], Pt[:, :10, :])

                chain(V[:, 9, :], V[:, 8, :])  # T10
                chain(V[:, 10, :], V[:, 7, :])  # T11
                chain(V[:, 13, :], V[:, 7, :])  # T14
                chain(V[:, 11:15:3, :], V[:, 10:14:3, :])  # T{12,15}
                chain(V[:, 12:16:3, :], V[:, 11:15:3, :])  # T{13,16}

                pf2 = Pt[:, 10:, :].rearrange("p a b -> p (a b)")
                nc.vector.tensor_add(
                    pf2, V[:, 9:16, :].rearrange("p a b -> p (a b)"), pf2
                )
                store_eng.dma_start(out.ap()[r0 : r0 + P, 10:], Pt[:, 10:, :])
            else:
                chain(V[:, 0, :], t0[:])  # T1
                chain(V[:, 3, :], t0[:])  # T4
                chain(V[:, 6, :], t0[:])  # T7
                chain(V[:, 1:8:3, :], V[:, 0:7:3, :])  # T{2,5,8}
                chain(V[:, 2:9:3, :], V[:, 1:8:3, :])  # T{3,6,9}
                chain(V[:, 9, :], V[:, 8, :])  # T10
                chain(V[:, 10, :], V[:, 7, :])  # T11
                chain(V[:, 13, :], V[:, 7, :])  # T14
                chain(V[:, 11:15:3, :], V[:, 10:14:3, :])  # T{12,15}
                chain(V[:, 12:16:3, :], V[:, 11:15:3, :])  # T{13,16}
                nc.vector.tensor_add(Pt[:, 0, :], t0[:], Pt[:, 0, :])
                pf = Pt[:, 1:, :].rearrange("p a b -> p (a b)")
                nc.vector.tensor_add(
                    pf, V[:, 0:16, :].rearrange("p a b -> p (a b)"), pf
                )
                store_eng.dma_start(out.ap()[r0 : r0 + P], Pt[:])

        if rep_ctx is not None:
            rep_ctx.__exit__(None, None, None)

    nc.compile()
    return nc


def _get_nc():
    global _nc_cache
    if _nc_cache is None:
        _nc_cache = _build()
    return _nc_cache


def _make_in_maps(points, vectors, attntion_scors, Fa):
    points = np.ascontiguousarray(points, dtype=np.float32)
    vectors = np.ascontiguousarray(vectors, dtype=np.float32)
    attn = np.ascontiguousarray(attntion_scors, dtype=np.float32)
    fa = np.float32(np.asarray(Fa).reshape(-1)[0])
    fa_pos = np.full((P, 1), fa, dtype=np.float32)
    fa_neg = np.full((P, 1), -fa, dtype=np.float32)
    iota = np.tile(np.arange(NJ, dtype=np.float32), (P, 1))
    in_maps = []
    for c in range(N_CORES):
        s = slice(c * B, (c + 1) * B)
        in_maps.append(
            {
                "points": points[s],
                "vectors": vectors[s],
                "attn": attn[s],
                "fa_pos": fa_pos,
                "fa_neg": fa_neg,
                "iota": iota,
            }
        )
    return in_maps


def run(points, vectors, attntion_scors, Fa, trace=False, **spmd_kwargs):
    nc = _get_nc()
    in_maps = _make_in_maps(points, vectors, attntion_scors, Fa)
    res = run_bass_kernel_spmd(
        nc, in_maps, core_ids=list(range(N_CORES)), trace=trace, **spmd_kwargs
    )
    full = np.concatenate([res.results[c]["out"] for c in range(N_CORES)], axis=0)
    return full, res


def kernel(points, vectors, attntion_scors, Fa):
    full, _ = run(points, vectors, attntion_scors, Fa)
    return full
